# revision 35
# baseline (speedup 1.0000x reference)
"""GQA attention block (QKV proj + RoPE + attention + out proj) on 8 TRN2 cores.

Sharding: tensor-parallel over heads. Each core gets 4 Q heads + their single
shared KV head (GQA groups intact), plus the matching Wo row-slice. Cores
produce partial [B*S, D] outputs that the host sums.

Per-core dataflow (all matmuls bf16, fp32 PSUM accumulate):
  - host pre-transposes x -> xT [B, D, S] so projections run as W.T @ x.T
    with head-dims on partitions; x is tiled chunk-major
    [B, NSS, KC, P, KTC, SS] so each (b, ss, kc) chunk is one contiguous
    0.5MB DMA and projection matmuls start as soon as the first chunk lands.
  - Q proj per head-pair: psum[128, 512] = sum_kt Wq[kt,128].T @ xT[kt,512];
    bias fused into a DVE tensor_scalar_add psum->sbuf copy (keeps ScalarE
    free for exp); RoPE (split-half layout, host permutes Wq/Wk columns so
    rotation halves are contiguous rows) on DVE, once per (b, pair).
  - K+V packed in one projection (K rows 0-63, V rows 64-127).
  - scoresT[t,s] for a head pair land in ONE 2-bank psum tile [128, 1024]
    via row-packed K=64 matmuls (head A rows 0-63 -> cols 0:512, head B rows
    64-127 -> cols 512:1024); ONE exp [128, 1024] per t-tile amortizes the
    ScalarE per-instruction overhead.
  - AV: lhsT = [ones x 64 | v] so psum rows 0-63 accumulate the softmax
    denominator (replicated) and rows 64-127 o.T; normalize with
    reciprocal_approx_fast (base-0 only!) + multiply on the way to SBUF.
  - O proj: psum[s,e] = sum oT[128,s].T @ Wo[128,e]; copy to SBUF; DMA out.
  - attention rounds are ScalarE-bound (exp 1077ns vs 852ns of PE work per
    t-tile), and the PE queue is in-order, so each attention phase PUMPS a
    filler generator (the next batch's projections / b0's output projection)
    between t-tiles to keep the PE busy during exp waits. Emission order:
      kvqp0(0) | attn(0,0)+qp1(0) | attn(0,1)+kvqp0(1) | attn(1,0)+qp1(1)
      | attn(1,1)+oproj(0)+oproj(1,ss) tails.
"""

import sys

sys.path.insert(0, "/opt/trn_rl_repo")

from contextlib import ExitStack

import numpy as np
import ml_dtypes

import concourse.bass as bass  # noqa: F401
import concourse.tile as tile
from concourse import bacc, mybir
from concourse.bass_utils import run_bass_kernel_spmd

BF16 = mybir.dt.bfloat16
F32 = mybir.dt.float32
F16 = mybir.dt.float16
AF = mybir.ActivationFunctionType

B, S, D = 2, 2048, 2048
QH, KVH, HD = 32, 8, 64
NCORES = 8
QH_LOC = QH // NCORES  # 4 q-heads per core
P = 128
SS = 512  # s-slice (psum free dim)
NSS = S // SS  # 4
KT = D // P  # 16 contraction tiles for projections
KC = 4  # x DMA chunks per (b, ss)
KTC = KT // KC  # 4 contraction tiles per chunk
NT = S // P  # 16 t-tiles for attention
NPAIR = QH_LOC // 2  # 2 head-pairs per core
SCALE = 1.0 / float(np.sqrt(HD))

# within-head dim permutation: even dims (cos half) first, odd dims second
_PERM = np.concatenate([np.arange(0, HD, 2), np.arange(1, HD, 2)])

DEBUG_DUMPS = False

_SENTINEL = object()


class Pump:
    """Pulls units from a chain of emission generators to fill PE queue
    bubbles. Generators can be appended mid-phase (rolling oproj tails)."""

    def __init__(self, *gens):
        self.gens = [g for g in gens if g is not None]

    def append(self, gen):
        self.gens.append(gen)

    def take(self, k):
        for _ in range(k):
            while self.gens:
                if next(self.gens[0], _SENTINEL) is _SENTINEL:
                    self.gens.pop(0)
                else:
                    break
            if not self.gens:
                return

    def drain(self):
        while self.gens:
            for _ in self.gens.pop(0):
                pass


def _rope(nc, tmp_pool, qsl, cos_sb, sin_sb, head_bases, cols):
    """In-place RoPE on qsl rows [hb, hb+64) for each hb (split-half layout).

    qsl covers sequence columns `cols` (a slice); the tables are indexed with
    the same columns. Both SBUF inputs of each tensor_tensor op must share a
    base partition (walrus verifier); tables are 32-row periodic so any
    aligned row block works.
    """
    width = cols.stop - cols.start
    t1 = tmp_pool.tile([P, width], BF16, tag="ropetmp1")
    t2 = tmp_pool.tile([P, width], BF16, tag="ropetmp2")
    for hb in head_bases:
        lo = slice(hb, hb + 32)
        hi = slice(hb + 32, hb + 64)
        x0 = qsl[lo]
        x1 = qsl[hi]
        nc.vector.tensor_mul(t1[lo], x0, cos_sb[lo, cols])  # x0*cos @ lo
        nc.vector.tensor_mul(t2[lo], x1, sin_sb[hi, cols])  # x1*sin -> lo
        nc.vector.tensor_mul(t1[hi], x0, sin_sb[lo, cols])  # x0*sin -> hi
        nc.vector.tensor_mul(t2[hi], x1, cos_sb[hi, cols])  # x1*cos @ hi
        nc.vector.tensor_sub(x0, t1[lo], t2[lo])
        nc.vector.tensor_add(x1, t1[hi], t2[hi])


def build_nc():
    nc = bacc.Bacc("TRN2", target_bir_lowering=False, debug=False, num_devices=NCORES)

    xt_d = nc.dram_tensor("xt", [B, NSS, KC, P, KTC, SS], BF16, kind="ExternalInput")
    wq_d = nc.dram_tensor("wq", [P, KT, NPAIR * P], BF16, kind="ExternalInput")
    wkv_d = nc.dram_tensor("wkv", [P, KT, P], BF16, kind="ExternalInput")
    wo_d = nc.dram_tensor("wo", [P, 2, D], BF16, kind="ExternalInput")
    cos_d = nc.dram_tensor("cost", [P, S], BF16, kind="ExternalInput")
    sin_d = nc.dram_tensor("sint", [P, S], BF16, kind="ExternalInput")
    bq_d = nc.dram_tensor("bq", [P, NPAIR], F32, kind="ExternalInput")
    bkv_d = nc.dram_tensor("bkv", [P, 1], F32, kind="ExternalInput")
    out_d = nc.dram_tensor("out", [B * S, D], F16, kind="ExternalOutput")
    if DEBUG_DUMPS:
        dqa_d = nc.dram_tensor("dqa", [P, B, NPAIR, S], BF16, kind="ExternalOutput")
        dkv_d = nc.dram_tensor("dkv", [P, B, S], BF16, kind="ExternalOutput")
        dvaug_d = nc.dram_tensor("dvaug", [P, B, NT, P], BF16, kind="ExternalOutput")
        dot_d = nc.dram_tensor("dot", [P, B, 2, S], BF16, kind="ExternalOutput")

    with tile.TileContext(nc) as tc:
        with ExitStack() as ctx:
            consts = ctx.enter_context(tc.tile_pool(name="consts", bufs=1))
            acts = ctx.enter_context(tc.tile_pool(name="acts", bufs=1))
            xpool = ctx.enter_context(tc.tile_pool(name="xt", bufs=16))
            tmp_pool = ctx.enter_context(tc.tile_pool(name="tmp", bufs=2))
            ppool = ctx.enter_context(tc.tile_pool(name="pexp", bufs=6))
            rpool = ctx.enter_context(tc.tile_pool(name="recip", bufs=2))
            opool = ctx.enter_context(tc.tile_pool(name="osb", bufs=3))
            # PSUM: scores 2x[128,1024] (4 banks) + av 2x[128,512] (2) +
            # proj/fin shared 2x[128,512] (2) = 8 banks exactly.
            sc_ps = ctx.enter_context(tc.tile_pool(name="sc", bufs=2, space="PSUM"))
            av_ps = ctx.enter_context(tc.tile_pool(name="av", bufs=2, space="PSUM"))
            pf_ps = ctx.enter_context(tc.tile_pool(name="pf", bufs=2, space="PSUM"))

            # ---- resident constants (DMA order matters: wkv + the first b0
            # x chunk are the startup critical path; wq is first used ~4us
            # in, cos/sin ~10us, wo ~300us) ----
            wkv_sb = consts.tile([P, KT, P], BF16)
            nc.sync.dma_start(wkv_sb[:], wkv_d.ap())
            bkv_sb = consts.tile([P, 1], F32)
            bq_sb = consts.tile([P, NPAIR], F32)
            wq_sb = consts.tile([P, KT, NPAIR * P], BF16)
            cos_sb = consts.tile([P, S], BF16)
            sin_sb = consts.tile([P, S], BF16)
            wo_sb = consts.tile([P, 2, D], BF16)
            # ---- persistent activations ----
            qa_sb = acts.tile([P, B, NPAIR, S], BF16)  # rotated q, pair tiles
            kv_sb = acts.tile([P, B, S], BF16)  # rows 0-63 k(rot), 64-127 v
            kk_sb = acts.tile([P, B, S], BF16)  # rows 64-127 = copy of k
            vaug_sb = acts.tile([P, B, NT, P], BF16)  # [t, 0:64]=1, [64:128]=v
            ot_sb = acts.tile([P, B, 2, S], BF16)  # normalized o.T stacked

            nc.any.memset(vaug_sb[:, :, :, 0:HD], 1.0)

            def proj16(ps, w, xc):
                """16 accumulating matmuls, chunk-ordered: psum [128, SS]."""
                for kc in range(KC):
                    for kt in range(KTC):
                        k = kc * KTC + kt
                        nc.tensor.matmul(
                            ps[:],
                            w[:, k],
                            xc[kc][:, kt],
                            start=(k == 0),
                            stop=(k == KT - 1),
                        )
                        yield

            def emit_chunks(b, ss, xts):
                chunks = []
                for kc in range(KC):
                    t = xpool.tile([P, KTC, SS], BF16, tag="xt")
                    nc.sync.dma_start(t[:], xt_d.ap()[b, ss, kc])
                    chunks.append(t)
                xts.append(chunks)

            def kvqp0_gen(b, xts):
                """KV + pair-0 Q projections for batch b as a unit generator.

                For b0: only ss0/ss1 x chunks + the constants needed early
                are DMA'd upfront; ss2/ss3 chunks are emitted after the
                ss0/ss1 compute units so the vaug transposes of ss0/ss1 sit
                EARLY in the sync queue (AV round 0 needs them, and the
                in-order PE queue blocks on it). cos/sin's first 512 cols
                load right after wkv so the rope chain (the DVE-side gate
                for the first scores matmul) starts as soon as the kv-ss0
                psum copy lands. For b1 everything is upfront (not
                latency-critical; streamed during attn(0,1))."""
                if b == 0:
                    nc.sync.dma_start(cos_sb[:, 0:SS], cos_d.ap()[:, 0:SS])
                    nc.sync.dma_start(sin_sb[:, 0:SS], sin_d.ap()[:, 0:SS])
                    emit_chunks(b, 0, xts)
                    nc.sync.dma_start(bkv_sb[:], bkv_d.ap())
                    nc.sync.dma_start(bq_sb[:], bq_d.ap())
                    nc.sync.dma_start(wq_sb[:, :, 0:P], wq_d.ap()[:, :, 0:P])
                    emit_chunks(b, 1, xts)
                    nc.sync.dma_start(cos_sb[:, SS:S], cos_d.ap()[:, SS:S])
                    nc.sync.dma_start(sin_sb[:, SS:S], sin_d.ap()[:, SS:S])
                    emit_chunks(b, 2, xts)
                    nc.sync.dma_start(
                        wq_sb[:, :, P : 2 * P], wq_d.ap()[:, :, P : 2 * P]
                    )
                    emit_chunks(b, 3, xts)
                    nc.sync.dma_start(wo_sb[:], wo_d.ap())
                else:
                    for ss in range(NSS):
                        emit_chunks(b, ss, xts)
                for ss in range(NSS):
                    sl = slice(ss * SS, (ss + 1) * SS)
                    ps = pf_ps.tile([P, SS], F32, tag="pf")
                    yield from proj16(ps, wkv_sb, xts[ss])
                    nc.vector.tensor_scalar_add(kv_sb[:, b, sl], ps[:], bkv_sb[:])
                    yield
                    # rope k + kk copy + vaug transposes FIRST (they gate the
                    # scores/AV of attention block ss0), then the q path.
                    _rope(nc, tmp_pool, kv_sb[:, b, sl], cos_sb, sin_sb, (0,), sl)
                    yield
                    nc.vector.tensor_copy(kk_sb[HD:P, b, sl], kv_sb[0:HD, b, sl])
                    yield
                    # b0's vaug transposes ride the ScalarE hwdge queue: the
                    # sync queue is streaming x for ~30us, and AV of the
                    # first attention block needs vaug early (in-order PE
                    # queue). ScalarE is idle until the first exp (~21us).
                    dma_eng = nc.scalar if b == 0 else nc.sync
                    for ci in range(ss * (SS // P), (ss + 1) * (SS // P)):
                        csl = slice(ci * P, (ci + 1) * P)
                        dma_eng.dma_start_transpose(
                            vaug_sb[:, b, ci, HD:P], kv_sb[HD:P, b, csl]
                        )
                    yield
                    ps = pf_ps.tile([P, SS], F32, tag="pf")
                    yield from proj16(ps, wq_sb[:, :, 0:P], xts[ss])
                    nc.vector.tensor_scalar_add(
                        qa_sb[:, b, 0, sl], ps[:], bq_sb[:, 0:1]
                    )
                    yield
                    _rope(
                        nc, tmp_pool, qa_sb[:, b, 0, sl], cos_sb, sin_sb,
                        (0, HD), sl,
                    )
                    yield
                    if ss == 0:
                        # first s-slice fully projected + roped: attention
                        # rounds tt 0-3 are unblocked from here.
                        yield "first"

            def qp1_gen(b, xts):
                for ss in range(NSS):
                    sl = slice(ss * SS, (ss + 1) * SS)
                    ps = pf_ps.tile([P, SS], F32, tag="pf")
                    yield from proj16(ps, wq_sb[:, :, P : 2 * P], xts[ss])
                    nc.vector.tensor_scalar_add(
                        qa_sb[:, b, 1, sl], ps[:], bq_sb[:, 1:2]
                    )
                    yield
                    if ss % 2 == 1:
                        hl = slice((ss - 1) * SS, (ss + 1) * SS)
                        _rope(
                            nc, tmp_pool, qa_sb[:, b, 1, hl], cos_sb, sin_sb,
                            (0, HD), hl,
                        )
                        yield

            def attn_pair(b, pair, pump=None, per_round=1, rates=None, tail=None):
                    pump = pump or Pump()
                    for ss in range(NSS):
                        sl = slice(ss * SS, (ss + 1) * SS)
                        po0 = av_ps.tile([P, SS], F32, tag="av")
                        po1 = av_ps.tile([P, SS], F32, tag="av")
                        for tt in range(NT):
                            csl = slice(tt * P, (tt + 1) * P)
                            # both heads' scoresT in one 2-bank psum tile
                            sc = sc_ps.tile([P, 2 * SS], F32, tag="sc")
                            nc.tensor.matmul(
                                sc[:, 0:SS],
                                kv_sb[0:HD, b, csl],
                                qa_sb[0:HD, b, pair, sl],
                                start=True,
                                stop=True,
                            )
                            nc.tensor.matmul(
                                sc[:, SS : 2 * SS],
                                kk_sb[HD:P, b, csl],
                                qa_sb[HD:P, b, pair, sl],
                                start=True,
                                stop=True,
                                tile_position=(HD, 0),
                            )
                            pa = ppool.tile([P, 2 * SS], BF16, tag="p")
                            nc.scalar.activation(pa[:], sc[:], AF.Exp, scale=SCALE)
                            nc.tensor.matmul(
                                po0[:],
                                vaug_sb[:, b, tt],
                                pa[:, 0:SS],
                                start=(tt == 0),
                                stop=(tt == NT - 1),
                            )
                            nc.tensor.matmul(
                                po1[:],
                                vaug_sb[:, b, tt],
                                pa[:, SS : 2 * SS],
                                start=(tt == 0),
                                stop=(tt == NT - 1),
                            )
                            pump.take(rates(ss, tt) if rates else per_round)
                        # normalize: rows 0-63 hold sumexp (replicated) at
                        # base 0, where reciprocal_approx_fast works; o.T is
                        # in rows 64-127.
                        r0 = rpool.tile([HD, SS], F32, tag="r")
                        r1 = rpool.tile([HD, SS], F32, tag="r")
                        nc.vector.reciprocal_approx_fast(r0[:], po0[0:HD])
                        nc.vector.reciprocal_approx_fast(r1[:], po1[0:HD])
                        nc.vector.tensor_mul(
                            ot_sb[0:HD, b, pair, sl], po0[HD:P], r0[:]
                        )
                        nc.vector.tensor_mul(
                            ot_sb[HD:P, b, pair, sl], po1[HD:P], r1[:]
                        )
                        if tail is not None:
                            tail(ss)
                    pump.drain()

            def oproj_fine(b, sc_lo, sc_hi, act_copies=False, alt_pool=False):
                """O-projection units for s-chunks [sc_lo, sc_hi).

                act_copies: route half the psum->sbuf copies through ScalarE
                (only for the final tail, when the exp stream is done).
                alt_pool: alternate pf/av psum pools for 4-deep pipelining
                (only for the final tail, when attention has released av)."""
                for sc_i in range(sc_lo, sc_hi):
                    scl = slice(sc_i * P, (sc_i + 1) * P)
                    row = slice(b * S + sc_i * P, b * S + (sc_i + 1) * P)
                    ob = opool.tile([P, D], F16, tag="osb")
                    for es in range(NSS):
                        esl = slice(es * SS, (es + 1) * SS)
                        if alt_pool and es % 2 == 0:
                            pf = av_ps.tile([P, SS], F32, tag="av")
                        else:
                            pf = pf_ps.tile([P, SS], F32, tag="pf")
                        for kt2 in range(2):
                            nc.tensor.matmul(
                                pf[:],
                                ot_sb[:, b, kt2, scl],
                                wo_sb[:, kt2, esl],
                                start=(kt2 == 0),
                                stop=(kt2 == 1),
                            )
                        if act_copies and es >= 2:
                            nc.scalar.activation(ob[:, esl], pf[:], AF.Identity)
                        else:
                            nc.vector.tensor_copy(ob[:, esl], pf[:])
                        yield
                        if es == 1:
                            # stream the first half while the second computes
                            nc.sync.dma_start(
                                out_d.ap()[row, 0 : 2 * SS], ob[:, 0 : 2 * SS]
                            )
                    nc.sync.dma_start(
                        out_d.ap()[row, 2 * SS : D], ob[:, 2 * SS : D]
                    )
                    yield

            # prologue: b0 KV + Q-pair0 projections for the FIRST s-slice,
            # drained inline (the x chunk DMAs pace the matmuls); slices
            # ss 1-3 are pumped at 10 units/round during attention rounds
            # tt 0-11 (round tt only touches key t-tiles up to tt*128).
            xts0, xts1 = [], []
            kvq0 = kvqp0_gen(0, xts0)
            for v in kvq0:
                if v == "first":
                    break
            # attention phases with filler pumps (see module docstring).
            # NOTE: Pump chains run generators strictly one at a time so two
            # pf-psum users never interleave their open accumulation tiles.
            attn_pair(
                0, 0, pump=Pump(kvq0, qp1_gen(0, xts0)),
                rates=lambda ss, tt: 10 if (ss == 0 and tt < 12) else 1,
            )
            attn_pair(0, 1, pump=Pump(kvqp0_gen(1, xts1)), per_round=2)
            attn_pair(1, 0, pump=Pump(qp1_gen(1, xts1)), per_round=1)

            p11 = Pump(oproj_fine(0, 0, NT))

            def tail11(ss):
                if ss < NSS - 1:
                    # roll this block's O-proj into the next block's filler
                    p11.append(oproj_fine(1, 4 * ss, 4 * ss + 4))
                else:
                    p11.drain()
                    for _ in oproj_fine(
                        1, 4 * ss, 4 * ss + 4, act_copies=True, alt_pool=True
                    ):
                        pass

            # drop to 1 take/round in the last block so rolling oproj units
            # remain to cover the final normalize window before the tail.
            attn_pair(
                1, 1, pump=p11, rates=lambda ss, tt: 1 if ss == 3 else 2,
                tail=tail11,
            )

            if DEBUG_DUMPS:
                nc.sync.dma_start(dqa_d.ap(), qa_sb[:])
                nc.sync.dma_start(dkv_d.ap(), kv_sb[:])
                nc.sync.dma_start(dvaug_d.ap(), vaug_sb[:])
                nc.sync.dma_start(dot_d.ap(), ot_sb[:])

    nc.compile()
    return nc


_NC_CACHE = None


def _get_nc():
    global _NC_CACHE
    if _NC_CACHE is None:
        _NC_CACHE = build_nc()
    return _NC_CACHE


def prepare_in_maps(x, freqs, Wq, bq, Wk, bk, Wv, bv, Wo, bo):
    x = np.asarray(x, np.float32)
    freqs = np.asarray(freqs, np.float32)
    Wq = np.asarray(Wq, np.float32)
    bq = np.asarray(bq, np.float32)
    Wk = np.asarray(Wk, np.float32)
    bk = np.asarray(bk, np.float32)
    Wv = np.asarray(Wv, np.float32)
    bv = np.asarray(bv, np.float32)
    Wo = np.asarray(Wo, np.float32)

    bf = ml_dtypes.bfloat16
    # [B, S, D] -> [B, D, S] -> tiled [B, NSS, KC, P(p), KTC(o), SS] with
    # d = (kc*KTC + o)*P + p and s = ss*SS + j, so each (b, ss, kc) DMA is
    # one contiguous 0.5MB chunk.
    xt = (
        x.transpose(0, 2, 1)
        .reshape(B, KC, KTC, P, NSS, SS)
        .transpose(0, 4, 1, 3, 2, 5)
    )
    xt = np.ascontiguousarray(xt).astype(bf)
    cost = np.ascontiguousarray(np.tile(freqs[:, :, 0].T, (4, 1))).astype(bf)
    sint = np.ascontiguousarray(np.tile(freqs[:, :, 1].T, (4, 1))).astype(bf)

    in_maps = []
    for c in range(NCORES):
        hq = slice(c * QH_LOC * HD, (c + 1) * QH_LOC * HD)
        hk = slice(c * HD, (c + 1) * HD)
        wq_c = Wq[:, hq].reshape(D, QH_LOC, HD)[:, :, _PERM].reshape(D, QH_LOC * HD)
        bq_c = bq[hq].reshape(QH_LOC, HD)[:, _PERM].reshape(NPAIR, P).T
        wk_c = Wk[:, hk][:, _PERM]
        wv_c = Wv[:, hk]
        wkv_c = np.concatenate([wk_c, wv_c], axis=1)
        bkv_c = np.concatenate([bk[hk][_PERM], bv[hk]])[:, None]
        wo_c = Wo[hq, :]
        in_maps.append(
            {
                "xt": xt,
                "wq": np.ascontiguousarray(
                    wq_c.reshape(KT, P, NPAIR * P).transpose(1, 0, 2)
                ).astype(bf),
                "wkv": np.ascontiguousarray(
                    wkv_c.reshape(KT, P, P).transpose(1, 0, 2)
                ).astype(bf),
                "wo": np.ascontiguousarray(
                    wo_c.reshape(2, P, D).transpose(1, 0, 2)
                ).astype(bf),
                "cost": cost,
                "sint": sint,
                "bq": np.ascontiguousarray(bq_c, dtype=np.float32),
                "bkv": np.ascontiguousarray(bkv_c, dtype=np.float32),
            }
        )
    return in_maps


def run(in_maps, trace=False, **kw):
    nc = _get_nc()
    return run_bass_kernel_spmd(nc, in_maps, list(range(NCORES)), trace=trace, **kw)


def kernel(**inputs):
    in_maps = prepare_in_maps(**{k: inputs[k] for k in (
        "x", "freqs", "Wq", "bq", "Wk", "bk", "Wv", "bv", "Wo", "bo")})
    res = run(in_maps, trace=False)
    acc = np.zeros((B * S, D), np.float64)
    for r in res.results:
        acc += r["out"].astype(np.float64)
    out = acc.astype(np.float32) + np.asarray(inputs["bo"], np.float32)[None, :]
    return out.reshape(B, S, D)
